# revision 5
# baseline (speedup 1.0000x reference)
"""Trainium2 Bass kernel for nn_MultiHeadAttention_44908178047033.

T5-style MHA (relative-position bias, bidirectional) over
B=2, L=2048, D=768, H=12, DK=64.

Sharding: 8 cores = 2 batches x 4 head-groups (3 heads each).
Each core computes Q/K/V projections for its (batch, 3 heads), fused
transposed-orientation attention (scores kept as S^T [k, q] so the
softmax denominator and the PV contraction both run as PE matmuls
without transposing the probability matrix), and a partial output
projection. Host sums the 4 per-head-group partials per batch.

v2 (perf rework over the f32r baseline):
- all activations/weights cast to bf16 on host: halves input DMA and
  runs every matmul at bf16 rate
- loop restructured q-half-major so only 2 PV accumulator banks are
  live per head: heads 0/1 run interleaved (their score matmuls sit on
  partition halves 0-63/64-127, so consecutive issues share the PE
  array via row-group concurrency); head 2's two q-block matmuls are
  paired the same way through duplicated K/Q rows
- single ACT table preload (natural_log_exp_and_others) so the
  Ln/Exp softmax normalization never swaps activation tables
- output projection of q-half 0 interleaves with q-half 1's attention;
  partial outputs returned as bf16

Relative-position bias: the T5 bias f(k-q) is constant for |k-q| >= 128
(log-bucketing saturates), so
  exp(s + f) = exp(s + cm)            for k-q <= -128  (ACT bias, free)
             = exp(s + cp)            for k-q >= +128  (ACT bias, free)
             = exp(s + cm + (f - cm)) for |k-q| < 128  (DVE add from a
               host-precomputed per-partition shifted Toeplitz table,
               read with a negative free-dim stride)
"""

import math
import sys
import threading

import numpy as np

sys.path.insert(0, "/opt/trn_rl_repo")

B, L, D = 2, 2048, 768
H, DK = 12, 64
NUM_BUCKETS, MAX_DIST = 32, 128
HP = 3            # heads per core
HD = HP * DK      # 192 cols per head-group
NCORES = 8
KC = 16           # key chunks of 128
CCH = 6           # contraction chunks of 128 over D

_cache = {}
_lock = threading.Lock()


def _np_bucket(d):
    rel = d
    ret = np.zeros_like(rel)
    n = -rel
    nb = NUM_BUCKETS // 2
    ret = ret + (n < 0).astype(np.int32) * nb
    n = np.abs(n)
    mx = nb // 2
    is_small = n < mx
    n_safe = np.maximum(n, 1).astype(np.float32)
    vl = mx + (
        np.log(n_safe / mx) / math.log(MAX_DIST / mx) * (nb - mx)
    ).astype(np.int32)
    vl = np.minimum(vl, nb - 1)
    return ret + np.where(is_small, n, vl)


def _build_program():
    import concourse.bacc as bacc
    import concourse.bass as bass
    import concourse.mybir as mybir
    import concourse.tile as tile
    from concourse.hw_specs import get_activation_tables

    dt = mybir.dt
    f32, bf16 = dt.float32, dt.bfloat16
    Exp, Ln = mybir.ActivationFunctionType.Exp, mybir.ActivationFunctionType.Ln

    act_sets = list(get_activation_tables("gen3").keys())
    nle_id = act_sets.index("natural_log_exp_and_others")

    nc = bacc.Bacc("TRN2", target_bir_lowering=False, debug=False,
                   num_devices=NCORES)

    qT_d = nc.dram_tensor("qT", [D, L], bf16, kind="ExternalInput").ap()
    kvT_d = nc.dram_tensor("kvT", [D, L], bf16, kind="ExternalInput").ap()
    wqa_d = nc.dram_tensor("wqa", [D, 128], bf16, kind="ExternalInput").ap()
    wka_d = nc.dram_tensor("wka", [D, 128], bf16, kind="ExternalInput").ap()
    wqb_d = nc.dram_tensor("wqb", [D, 64], bf16, kind="ExternalInput").ap()
    wkb_d = nc.dram_tensor("wkb", [D, 64], bf16, kind="ExternalInput").ap()
    wv_d = nc.dram_tensor("wv", [D, HD], bf16, kind="ExternalInput").ap()
    wo_d = nc.dram_tensor("wo", [64, HP, D], bf16, kind="ExternalInput").ap()
    sh_d = nc.dram_tensor("sh", [HP, 128, 383], f32, kind="ExternalInput").ap()
    msk_d = nc.dram_tensor("msk", [128, KC], f32, kind="ExternalInput").ap()
    cm_d = nc.dram_tensor("cm", [128, HP], f32, kind="ExternalInput").ap()
    cp_d = nc.dram_tensor("cp", [128, HP], f32, kind="ExternalInput").ap()
    out_d = nc.dram_tensor("out_p", [L, D], bf16, kind="ExternalOutput").ap()

    with tile.TileContext(nc) as tc:
        with (
            tc.tile_pool(name="const", bufs=1) as cpool,
            tc.tile_pool(name="p", bufs=3) as ppool,
            tc.tile_pool(name="o", bufs=2) as opool,
            tc.tile_pool(name="nrm", bufs=4) as npool,
            tc.tile_pool(name="sp", bufs=2, space="PSUM") as sp,
            tc.tile_pool(name="pp", bufs=4, space="PSUM") as pp,
        ):
            # ---- persistent SBUF ----
            wqa = cpool.tile([128, CCH, 128], bf16, tag="wqa")
            wka = cpool.tile([128, CCH, 128], bf16, tag="wka")
            wqb = cpool.tile([128, CCH, 64], bf16, tag="wqb")
            wkb = cpool.tile([128, CCH, 64], bf16, tag="wkb")
            wv = cpool.tile([128, CCH, HD], bf16, tag="wv")
            wo = cpool.tile([64, HP, D], bf16, tag="wo")
            sh = cpool.tile([128, HP, 383], f32, tag="sh")
            msk = cpool.tile([128, KC], f32, tag="msk")
            cmc = cpool.tile([128, HP], f32, tag="cmc")
            cpc = cpool.tile([128, HP], f32, tag="cpc")
            qT = cpool.tile([128, CCH, L], bf16, tag="qT")
            kvT = cpool.tile([128, CCH, L], bf16, tag="kvT")
            # heads 0,1 stacked on partitions 0-63 / 64-127
            QTa = cpool.tile([128, L], bf16, tag="QTa")
            KTa = cpool.tile([128, L], bf16, tag="KTa")
            # head 2 duplicated on both partition halves (for jj-pairing)
            QTb = cpool.tile([128, L], bf16, tag="QTb")
            KTb = cpool.tile([128, L], bf16, tag="KTb")
            Vg = cpool.tile([128, KC, HP, 65], bf16, tag="Vg")
            AT = cpool.tile([64, HP, L], bf16, tag="AT")

            # single activation-table load covering both Exp and Ln; the
            # compile-time pass then sees every activation's table resident
            nc.scalar.add_instruction(mybir.InstLoadActFuncSet(
                name=nc.get_next_instruction_name(), ins=[], outs=[],
                act_func_set_id=nle_id))

            # ---- loads (weights first on the scalar HWDGE path so the
            # first projection matmuls can start early; bulk activations
            # stream on sync) ----
            nc.scalar.dma_start(out=wqa[:], in_=wqa_d.rearrange("(c p) n -> p c n", p=128))
            nc.scalar.dma_start(out=wka[:], in_=wka_d.rearrange("(c p) n -> p c n", p=128))
            nc.scalar.dma_start(out=wqb[:], in_=wqb_d.rearrange("(c p) n -> p c n", p=128))
            nc.scalar.dma_start(out=wkb[:], in_=wkb_d.rearrange("(c p) n -> p c n", p=128))
            nc.scalar.dma_start(out=wv[:], in_=wv_d.rearrange("(c p) n -> p c n", p=128))
            nc.scalar.dma_start(out=wo[:], in_=wo_d)
            nc.scalar.dma_start(out=sh[:], in_=sh_d.rearrange("h p y -> p h y"))
            nc.scalar.dma_start(out=msk[:], in_=msk_d)
            nc.scalar.dma_start(out=cmc[:], in_=cm_d)
            nc.scalar.dma_start(out=cpc[:], in_=cp_d)
            qT_r = qT_d.rearrange("(c p) n -> p c n", p=128)
            kvT_r = kvT_d.rearrange("(c p) n -> p c n", p=128)
            for src_r, dst in ((kvT_r, kvT), (qT_r, qT)):
                for c in range(CCH):
                    nc.sync.dma_start(out=dst[:, c, :], in_=src_r[:, c, :])

            # ---- Q/K projections ----
            # m-chunks: QTa = heads 0,1 of Q; KTa = heads 0,1 of K;
            # head 2 Q/K col-tiled into one PSUM tile (Q rows 0-63, K 64-127)
            for n in range(4):
                nsl = slice(512 * n, 512 * n + 512)
                for w_in, x_in, dst in ((wqa, qT, QTa), (wka, kvT, KTa)):
                    ps = pp.tile([128, 512], f32, tag="pp", name=f"ps{dst.name}_{n}")
                    for c in range(CCH):
                        nc.tensor.matmul(
                            ps[:], lhsT=w_in[:, c, :], rhs=x_in[:, c, nsl],
                            start=(c == 0), stop=(c == CCH - 1),
                        )
                    nc.vector.tensor_copy(dst[:, nsl], ps[:])
                # head-2 Q and K col-tiled onto array halves; separate PSUM
                # banks so the two accumulation groups' has_written clears
                # don't interfere
                psq = pp.tile([128, 512], f32, tag="pp", name=f"psbq_{n}")
                psk = pp.tile([128, 512], f32, tag="pp", name=f"psbk_{n}")
                for c in range(CCH):
                    nc.tensor.matmul(
                        psq[0:64, :], lhsT=wqb[:, c, :], rhs=qT[:, c, nsl],
                        start=(c == 0), stop=(c == CCH - 1),
                    )
                    nc.tensor.matmul(
                        psk[64:128, :], lhsT=wkb[:, c, :], rhs=kvT[:, c, nsl],
                        start=(c == 0), stop=(c == CCH - 1),
                        tile_position=(0, 64),
                    )
                nc.vector.tensor_copy(QTb[0:64, nsl], psq[0:64, :])
                nc.vector.tensor_copy(KTb[64:128, nsl], psk[64:128, :])
            # duplicate head-2 Q/K onto the other partition half
            nc.sync.dma_start(out=QTb[64:128, :], in_=QTb[0:64, :])
            nc.sync.dma_start(out=KTb[0:64, :], in_=KTb[64:128, :])

            # ---- V projection -> V_aug (bf16) with mask column ----
            for kc in range(KC):
                ps_v = pp.tile([128, 512], f32, tag="pp", name=f"psv{kc}")
                for c in range(CCH):
                    nc.tensor.matmul(
                        ps_v[:, 0:HD],
                        lhsT=kvT[:, c, 128 * kc:128 * kc + 128],
                        rhs=wv[:, c, :],
                        start=(c == 0), stop=(c == CCH - 1),
                    )
                nc.vector.tensor_copy(
                    Vg[:, kc, :, 0:64],
                    ps_v[:, 0:HD].rearrange("p (h d) -> p h d", h=HP))
                mrep = bass.AP(msk[:].tensor, msk[:].offset + kc,
                               [list(msk[:].ap[0]), [0, HP], [1, 1]])
                nc.vector.tensor_copy(Vg[:, kc, :, 64:65], mrep)

            def band_add(s, h, kc, ha):
                """near-diagonal bias add (in place, PSUM); s covers
                columns [ha, ha+1024)."""
                qlo = max(0, 128 * kc - 128)
                qhi = min(L, 128 * kc + 255)
                x0 = (2047 + 128 * kc - qlo) - 1793
                a = max(qlo, ha)
                b = min(qhi, ha + 1024)
                if b > a:
                    sh_ap = sh[:, h, :]
                    rev = bass.AP(
                        sh_ap.tensor, sh_ap.offset + x0 - (a - qlo),
                        [list(sh_ap.ap[0]), [-1, b - a]],
                    )
                    nc.vector.tensor_add(
                        s[:, a - ha:b - ha], s[:, a - ha:b - ha], rev)

            def exp_split(s, h, kc, ha, name):
                """exp with region-split bias: cp for q < wcp, cm after."""
                p = ppool.tile([128, 1024], bf16, tag="p", name=name)
                wcp = max(0, 128 * kc - 128)
                wl = min(max(wcp - ha, 0), 1024)
                if wl > 0:
                    nc.scalar.activation(
                        p[:, 0:wl], s[:, 0:wl], Exp,
                        bias=cpc[:, h:h + 1], scale=1.0)
                if wl < 1024:
                    nc.scalar.activation(
                        p[:, wl:1024], s[:, wl:1024], Exp,
                        bias=cmc[:, h:h + 1], scale=1.0)
                return p

            def normalize(pvs, h, qh):
                """pvs: two [65, 512] PSUM accumulators (numerator rows
                0-63, denominator row 64) -> AT[:, h, qh*1024 : +1024]."""
                for j in range(2):
                    pvsb = npool.tile([65, 512], bf16, tag="pvsb",
                                      name=f"pvsb{qh}_{h}_{j}")
                    nc.vector.tensor_copy(pvsb[:], pvs[j][:])
                    lns = npool.tile([1, 512], f32, tag="lns",
                                     name=f"l{qh}_{h}_{j}")
                    nc.scalar.activation(lns[:], pvsb[64:65, :], Ln)
                    inv = npool.tile([1, 512], bf16, tag="inv",
                                     name=f"i{qh}_{h}_{j}")
                    nc.scalar.activation(inv[:], lns[:], Exp, scale=-1.0)
                    invb = npool.tile([64, 512], bf16, tag="invb",
                                      name=f"ib{qh}_{h}_{j}")
                    nc.gpsimd.partition_broadcast(invb[:], inv[:])
                    qsl = slice(1024 * qh + 512 * j, 1024 * qh + 512 * j + 512)
                    nc.vector.tensor_mul(AT[:, h, qsl], pvsb[0:64, :], invb[:])

            def out_proj_chunk(qc):
                """output projection for query rows [128*qc, 128*qc+128)."""
                o = opool.tile([128, D], bf16, tag="o", name=f"o{qc}")
                for nlo, nw in ((0, 512), (512, 256)):
                    ps_o = pp.tile([128, 512], f32, tag="pp",
                                   name=f"po{qc}_{nlo}")
                    for h in range(HP):
                        nc.tensor.matmul(
                            ps_o[:, 0:nw],
                            lhsT=AT[:, h, 128 * qc:128 * qc + 128],
                            rhs=wo[:, h, nlo:nlo + nw],
                            start=(h == 0), stop=(h == HP - 1),
                        )
                    nc.vector.tensor_copy(o[:, nlo:nlo + nw], ps_o[:, 0:nw])
                nc.sync.dma_start(
                    out=out_d[128 * qc:128 * qc + 128, :], in_=o[:])

            # ---- fused attention, transposed orientation, q-half major ----
            for qh in range(2):
                ha = 1024 * qh
                # heads 0 and 1 interleaved: their score matmuls live on
                # partition halves 0-63 / 64-127 -> concurrent row groups
                pvs01 = [[pp.tile([65, 512], f32, tag="pp",
                                  name=f"pv{qh}_{h}_{j}") for j in range(2)]
                         for h in range(2)]
                for kc in range(KC):
                    ss = [sp.tile([128, 1024], f32, tag="sp",
                                  name=f"s{qh}_{h}_{kc}") for h in range(2)]
                    for jj in range(2):
                        qsl = slice(ha + 512 * jj, ha + 512 * jj + 512)
                        osl = slice(512 * jj, 512 * jj + 512)
                        nc.tensor.matmul(
                            ss[0][:, osl],
                            lhsT=KTa[0:64, 128 * kc:128 * kc + 128],
                            rhs=QTa[0:64, qsl], start=True, stop=True)
                        nc.tensor.matmul(
                            ss[1][:, osl],
                            lhsT=KTa[64:128, 128 * kc:128 * kc + 128],
                            rhs=QTa[64:128, qsl], start=True, stop=True)
                    for h in range(2):
                        band_add(ss[h], h, kc, ha)
                        p = exp_split(ss[h], h, kc, ha, f"p{qh}_{h}_{kc}")
                        for jj in range(2):
                            nc.tensor.matmul(
                                pvs01[h][jj][:],
                                lhsT=Vg[:, kc, h, :],
                                rhs=p[:, 512 * jj:512 * jj + 512],
                                start=(kc == 0), stop=(kc == KC - 1))
                for h in range(2):
                    normalize(pvs01[h], h, qh)

                # head 2: the two q-blocks run as paired row-group matmuls
                # through the duplicated K/Q partition halves
                pvs2 = [pp.tile([65, 512], f32, tag="pp",
                                name=f"pv{qh}_2_{j}") for j in range(2)]
                for kc in range(KC):
                    s2 = sp.tile([128, 1024], f32, tag="sp",
                                 name=f"s{qh}_2_{kc}")
                    nc.tensor.matmul(
                        s2[:, 0:512],
                        lhsT=KTb[0:64, 128 * kc:128 * kc + 128],
                        rhs=QTb[0:64, ha:ha + 512], start=True, stop=True)
                    nc.tensor.matmul(
                        s2[:, 512:1024],
                        lhsT=KTb[64:128, 128 * kc:128 * kc + 128],
                        rhs=QTb[64:128, ha + 512:ha + 1024],
                        start=True, stop=True)
                    band_add(s2, 2, kc, ha)
                    p = exp_split(s2, 2, kc, ha, f"p{qh}_2_{kc}")
                    for jj in range(2):
                        nc.tensor.matmul(
                            pvs2[jj][:],
                            lhsT=Vg[:, kc, 2, :],
                            rhs=p[:, 512 * jj:512 * jj + 512],
                            start=(kc == 0), stop=(kc == KC - 1))
                    # interleave previous q-half's output projection into
                    # the PE stream while ACT works on head 2's exps
                    if qh == 1 and kc % 2 == 1:
                        out_proj_chunk(kc // 2)
                normalize(pvs2, 2, qh)

            # ---- remaining output projection ----
            for qc in range(8, L // 128):
                out_proj_chunk(qc)

    nc.compile()
    return nc


def _get_program():
    with _lock:
        if "nc" not in _cache:
            _cache["nc"] = _build_program()
        return _cache["nc"]


def _host_prep(core, query, key_value, key_padding_mask, Wq, Wk, Wv, Wo, rel_emb):
    import ml_dtypes

    bf16 = ml_dtypes.bfloat16
    b, g = core // 4, core % 4
    mask = key_padding_mask[b].astype(np.float32)
    kv = key_value[b] * mask[:, None]
    qT = np.ascontiguousarray(query[b].T).astype(bf16)
    kvT = np.ascontiguousarray(kv.T).astype(bf16)
    sl = slice(HD * g, HD * (g + 1))
    wq = np.ascontiguousarray(Wq[:, sl])
    wk = np.ascontiguousarray(Wk[:, sl]) * np.float32(DK ** -0.5)
    wv = np.ascontiguousarray(Wv[:, sl]).astype(bf16)
    wo = np.ascontiguousarray(
        Wo[sl].reshape(HP, 64, D).transpose(1, 0, 2)).astype(bf16)

    d = np.arange(-2047, 2048)
    buckets = _np_bucket(d)
    heads = [HP * g + i for i in range(HP)]
    t = rel_emb[buckets][:, heads].astype(np.float32)  # [4095, HP]
    cm = t[0]
    cp = t[-1]
    # sh[h, p, y] = t[y + 1793 + p, h] - cm[h]
    p_i = np.arange(128)[:, None]
    y_i = np.arange(383)[None, :]
    sh = np.ascontiguousarray(
        (t[y_i + 1793 + p_i] - cm[None, None, :]).transpose(2, 0, 1))
    msk = np.ascontiguousarray(mask.reshape(KC, 128).T)
    cmc = np.ascontiguousarray(np.broadcast_to(cm[None, :], (128, HP)))
    cpc = np.ascontiguousarray(np.broadcast_to(cp[None, :], (128, HP)))
    return {
        "qT": qT, "kvT": kvT,
        "wqa": np.ascontiguousarray(wq[:, 0:128]).astype(bf16),
        "wka": np.ascontiguousarray(wk[:, 0:128]).astype(bf16),
        "wqb": np.ascontiguousarray(wq[:, 128:192]).astype(bf16),
        "wkb": np.ascontiguousarray(wk[:, 128:192]).astype(bf16),
        "wv": wv, "wo": wo,
        "sh": sh.astype(np.float32), "msk": msk,
        "cm": cmc.astype(np.float32), "cp": cpc.astype(np.float32),
    }


def make_in_maps(**inputs):
    return [_host_prep(c, **inputs) for c in range(NCORES)]


def kernel(query, key_value, key_padding_mask, Wq, Wk, Wv, Wo, rel_emb,
           _results_hook=None, _run_kwargs=None):
    from concourse.bass_utils import run_bass_kernel_spmd

    inputs = dict(query=np.asarray(query), key_value=np.asarray(key_value),
                  key_padding_mask=np.asarray(key_padding_mask),
                  Wq=np.asarray(Wq, np.float32), Wk=np.asarray(Wk, np.float32),
                  Wv=np.asarray(Wv, np.float32), Wo=np.asarray(Wo, np.float32),
                  rel_emb=np.asarray(rel_emb, np.float32))
    nc = _get_program()
    in_maps = make_in_maps(**inputs)
    res = run_bass_kernel_spmd(nc, in_maps, core_ids=list(range(NCORES)),
                               **(_run_kwargs or {}))
    if _results_hook is not None:
        _results_hook(res)
    out = np.zeros((B, L, D), np.float32)
    for c in range(NCORES):
        out[c // 4] += res.results[c]["out_p"].astype(np.float32)
    return out


# revision 8
# speedup vs baseline: 1.0190x; 1.0190x over previous
"""Trainium2 Bass kernel for nn_MultiHeadAttention_44908178047033.

T5-style MHA (relative-position bias, bidirectional) over
B=2, L=2048, D=768, H=12, DK=64.

Sharding: 8 cores = 2 batches x 4 head-groups (3 heads each).
Each core computes Q/K/V projections for its (batch, 3 heads), fused
transposed-orientation attention (scores kept as S^T [k, q] so the
softmax denominator and the PV contraction both run as PE matmuls
without transposing the probability matrix), and a partial output
projection. Host sums the 4 per-head-group partials per batch.

v3 perf structure (from trace analysis of the f32r baseline and v2):
- everything bf16: halves input DMA, runs all matmuls at bf16 rate
- single ACT table preload (natural_log_exp_and_others) so the Ln/Exp
  softmax normalization never swaps activation tables mid-kernel
- q-half-major loop with sequential heads: only 2 PV accumulator banks
  and 2 double-buffered score tiles are live, leaving 2 PSUM banks for
  interleaved filler matmuls
- the PE HAM clock gate re-throttles to 1.2 GHz after any ~3.4us idle
  window and only re-warms after ~3.4us of continuous work, so the V
  projection is interleaved into q-half 0's attention and the output
  projection of q-half 0 into q-half 1's attention: the PE instruction
  queue never drains at phase transitions and stays at 2.4 GHz

Relative-position bias: the T5 bias f(k-q) is constant for |k-q| >= 128
(log-bucketing saturates), so
  exp(s + f) = exp(s + cm)            for k-q <= -128  (ACT bias, free)
             = exp(s + cp)            for k-q >= +128  (ACT bias, free)
             = exp(s + cm + (f - cm)) for |k-q| < 128  (DVE add from a
               host-precomputed per-partition shifted Toeplitz table,
               read with a negative free-dim stride)
"""

import math
import sys
import threading

import numpy as np

sys.path.insert(0, "/opt/trn_rl_repo")

B, L, D = 2, 2048, 768
H, DK = 12, 64
NUM_BUCKETS, MAX_DIST = 32, 128
HP = 3            # heads per core
HD = HP * DK      # 192 cols per head-group
NCORES = 8
KC = 16           # key chunks of 128
CCH = 6           # contraction chunks of 128 over D

_cache = {}
_lock = threading.Lock()


def _np_bucket(d):
    rel = d
    ret = np.zeros_like(rel)
    n = -rel
    nb = NUM_BUCKETS // 2
    ret = ret + (n < 0).astype(np.int32) * nb
    n = np.abs(n)
    mx = nb // 2
    is_small = n < mx
    n_safe = np.maximum(n, 1).astype(np.float32)
    vl = mx + (
        np.log(n_safe / mx) / math.log(MAX_DIST / mx) * (nb - mx)
    ).astype(np.int32)
    vl = np.minimum(vl, nb - 1)
    return ret + np.where(is_small, n, vl)


def _build_program():
    import concourse.bacc as bacc
    import concourse.bass as bass
    import concourse.mybir as mybir
    import concourse.tile as tile
    from concourse.hw_specs import get_activation_tables

    dt = mybir.dt
    f32, bf16 = dt.float32, dt.bfloat16
    Exp, Ln = mybir.ActivationFunctionType.Exp, mybir.ActivationFunctionType.Ln

    act_sets = list(get_activation_tables("gen3").keys())
    nle_id = act_sets.index("natural_log_exp_and_others")

    nc = bacc.Bacc("TRN2", target_bir_lowering=False, debug=False,
                   num_devices=NCORES)

    qT_d = nc.dram_tensor("qT", [D, L], bf16, kind="ExternalInput").ap()
    kvT_d = nc.dram_tensor("kvT", [D, L], bf16, kind="ExternalInput").ap()
    wqa_d = nc.dram_tensor("wqa", [D, 128], bf16, kind="ExternalInput").ap()
    wka_d = nc.dram_tensor("wka", [D, 128], bf16, kind="ExternalInput").ap()
    wqb_d = nc.dram_tensor("wqb", [D, 64], bf16, kind="ExternalInput").ap()
    wkb_d = nc.dram_tensor("wkb", [D, 64], bf16, kind="ExternalInput").ap()
    wv_d = nc.dram_tensor("wv", [D, HD], bf16, kind="ExternalInput").ap()
    wo_d = nc.dram_tensor("wo", [64, HP, D], bf16, kind="ExternalInput").ap()
    sh_d = nc.dram_tensor("sh", [HP, 128, 383], f32, kind="ExternalInput").ap()
    msk_d = nc.dram_tensor("msk", [128, KC], f32, kind="ExternalInput").ap()
    cm_d = nc.dram_tensor("cm", [128, HP], f32, kind="ExternalInput").ap()
    cp_d = nc.dram_tensor("cp", [128, HP], f32, kind="ExternalInput").ap()
    out_d = nc.dram_tensor("out_p", [L, D], bf16, kind="ExternalOutput").ap()

    with tile.TileContext(nc) as tc:
        with (
            tc.tile_pool(name="const", bufs=1) as cpool,
            tc.tile_pool(name="p", bufs=3) as ppool,
            tc.tile_pool(name="o", bufs=2) as opool,
            tc.tile_pool(name="nrm", bufs=4) as npool,
            tc.tile_pool(name="sp", bufs=2, space="PSUM") as sp,
            tc.tile_pool(name="pp", bufs=4, space="PSUM") as pp,
        ):
            # ---- persistent SBUF ----
            wqa = cpool.tile([128, CCH, 128], bf16, tag="wqa")
            wka = cpool.tile([128, CCH, 128], bf16, tag="wka")
            wqb = cpool.tile([128, CCH, 64], bf16, tag="wqb")
            wkb = cpool.tile([128, CCH, 64], bf16, tag="wkb")
            wv = cpool.tile([128, CCH, HD], bf16, tag="wv")
            wo = cpool.tile([64, HP, D], bf16, tag="wo")
            sh = cpool.tile([128, HP, 383], f32, tag="sh")
            msk = cpool.tile([128, KC], f32, tag="msk")
            cmc = cpool.tile([128, HP], f32, tag="cmc")
            cpc = cpool.tile([128, HP], f32, tag="cpc")
            qT = cpool.tile([128, CCH, L], bf16, tag="qT")
            kvT = cpool.tile([128, CCH, L], bf16, tag="kvT")
            # heads 0,1 stacked on partitions 0-63 / 64-127
            QTa = cpool.tile([128, L], bf16, tag="QTa")
            KTa = cpool.tile([128, L], bf16, tag="KTa")
            # head 2: K on partitions 64-127 (straight from its col-tiled
            # projection), Q on 0-63 then DMA-duplicated to 64-127 so both
            # score operands live on the same partition half
            QTb = cpool.tile([128, L], bf16, tag="QTb")
            KTb = cpool.tile([128, L], bf16, tag="KTb")
            Vg = cpool.tile([128, KC, HP, 65], bf16, tag="Vg")
            AT = cpool.tile([64, HP, L], bf16, tag="AT")

            # single activation-table load covering both Exp and Ln; the
            # compile-time pass then sees every activation's table resident
            nc.scalar.add_instruction(mybir.InstLoadActFuncSet(
                name=nc.get_next_instruction_name(), ins=[], outs=[],
                act_func_set_id=nle_id))

            # ---- loads (weights first on the scalar HWDGE path so the
            # first projection matmuls can start early; bulk activations
            # stream on sync) ----
            nc.scalar.dma_start(out=wqa[:], in_=wqa_d.rearrange("(c p) n -> p c n", p=128))
            nc.scalar.dma_start(out=wka[:], in_=wka_d.rearrange("(c p) n -> p c n", p=128))
            nc.scalar.dma_start(out=wqb[:], in_=wqb_d.rearrange("(c p) n -> p c n", p=128))
            nc.scalar.dma_start(out=wkb[:], in_=wkb_d.rearrange("(c p) n -> p c n", p=128))
            nc.scalar.dma_start(out=wv[:], in_=wv_d.rearrange("(c p) n -> p c n", p=128))
            nc.scalar.dma_start(out=wo[:], in_=wo_d)
            nc.scalar.dma_start(out=sh[:], in_=sh_d.rearrange("h p y -> p h y"))
            nc.scalar.dma_start(out=msk[:], in_=msk_d)
            nc.scalar.dma_start(out=cmc[:], in_=cm_d)
            nc.scalar.dma_start(out=cpc[:], in_=cp_d)
            qT_r = qT_d.rearrange("(c p) n -> p c n", p=128)
            kvT_r = kvT_d.rearrange("(c p) n -> p c n", p=128)
            for src_r, dst in ((kvT_r, kvT), (qT_r, qT)):
                for c in range(CCH):
                    nc.sync.dma_start(out=dst[:, c, :], in_=src_r[:, c, :])

            # ---- Q/K projections ----
            # m-chunks: QTa/KTa = heads 0,1; head 2's Q and K run col-tiled
            # in one pass (Q on array cols 0-63, K on 64-127), separate PSUM
            # banks so the accumulation groups' has_written clears stay apart
            for n in range(4):
                nsl = slice(512 * n, 512 * n + 512)
                for w_in, x_in, dst in ((wqa, qT, QTa), (wka, kvT, KTa)):
                    ps = pp.tile([128, 512], f32, tag="pp", name=f"ps{dst.name}_{n}")
                    for c in range(CCH):
                        nc.tensor.matmul(
                            ps[:], lhsT=w_in[:, c, :], rhs=x_in[:, c, nsl],
                            start=(c == 0), stop=(c == CCH - 1),
                        )
                    nc.vector.tensor_copy(dst[:, nsl], ps[:])
                psq = pp.tile([128, 512], f32, tag="pp", name=f"psbq_{n}")
                psk = pp.tile([128, 512], f32, tag="pp", name=f"psbk_{n}")
                for c in range(CCH):
                    nc.tensor.matmul(
                        psq[0:64, :], lhsT=wqb[:, c, :], rhs=qT[:, c, nsl],
                        start=(c == 0), stop=(c == CCH - 1),
                    )
                    nc.tensor.matmul(
                        psk[64:128, :], lhsT=wkb[:, c, :], rhs=kvT[:, c, nsl],
                        start=(c == 0), stop=(c == CCH - 1),
                        tile_position=(0, 64),
                    )
                nc.vector.tensor_copy(QTb[0:64, nsl], psq[0:64, :])
                nc.vector.tensor_copy(KTb[64:128, nsl], psk[64:128, :])
            # head-2 scores contract on partitions 64-127; put Q there too
            nc.sync.dma_start(out=QTb[64:128, :], in_=QTb[0:64, :])

            # mask column of V_aug, all key chunks at once
            mrep = bass.AP(msk[:].tensor, msk[:].offset,
                           [list(msk[:].ap[0]), [1, KC], [0, HP], [1, 1]])
            nc.vector.tensor_copy(Vg[:, :, :, 64:65], mrep)

            def v_proj(kc):
                """V projection chunk -> Vg[:, kc, :, 0:64] (interleaved
                into q-half 0's attention as PE filler work)."""
                ps_v = pp.tile([128, 512], f32, tag="pp", name=f"psv{kc}")
                for c in range(CCH):
                    nc.tensor.matmul(
                        ps_v[:, 0:HD],
                        lhsT=kvT[:, c, 128 * kc:128 * kc + 128],
                        rhs=wv[:, c, :],
                        start=(c == 0), stop=(c == CCH - 1),
                    )
                nc.vector.tensor_copy(
                    Vg[:, kc, :, 0:64],
                    ps_v[:, 0:HD].rearrange("p (h d) -> p h d", h=HP))

            def band_add(s, h, kc, ha):
                """near-diagonal bias add (in place, PSUM); s covers
                columns [ha, ha+1024)."""
                qlo = max(0, 128 * kc - 128)
                qhi = min(L, 128 * kc + 255)
                x0 = (2047 + 128 * kc - qlo) - 1793
                a = max(qlo, ha)
                b = min(qhi, ha + 1024)
                if b > a:
                    sh_ap = sh[:, h, :]
                    rev = bass.AP(
                        sh_ap.tensor, sh_ap.offset + x0 - (a - qlo),
                        [list(sh_ap.ap[0]), [-1, b - a]],
                    )
                    nc.vector.tensor_add(
                        s[:, a - ha:b - ha], s[:, a - ha:b - ha], rev)

            def exp_split(s, h, kc, ha, name):
                """exp with region-split bias: cp for q < wcp, cm after."""
                p = ppool.tile([128, 1024], bf16, tag="p", name=name)
                wcp = max(0, 128 * kc - 128)
                wl = min(max(wcp - ha, 0), 1024)
                if wl > 0:
                    nc.scalar.activation(
                        p[:, 0:wl], s[:, 0:wl], Exp,
                        bias=cpc[:, h:h + 1], scale=1.0)
                if wl < 1024:
                    nc.scalar.activation(
                        p[:, wl:1024], s[:, wl:1024], Exp,
                        bias=cmc[:, h:h + 1], scale=1.0)
                return p

            def normalize(pvs, h, qh):
                """pvs: two [65, 512] PSUM accumulators (numerator rows
                0-63, denominator row 64) -> AT[:, h, qh*1024 : +1024]."""
                for j in range(2):
                    pvsb = npool.tile([65, 512], bf16, tag="pvsb",
                                      name=f"pvsb{qh}_{h}_{j}")
                    nc.vector.tensor_copy(pvsb[:], pvs[j][:])
                    lns = npool.tile([1, 512], f32, tag="lns",
                                     name=f"l{qh}_{h}_{j}")
                    nc.scalar.activation(lns[:], pvsb[64:65, :], Ln)
                    inv = npool.tile([1, 512], bf16, tag="inv",
                                     name=f"i{qh}_{h}_{j}")
                    nc.scalar.activation(inv[:], lns[:], Exp, scale=-1.0)
                    invb = npool.tile([64, 512], bf16, tag="invb",
                                      name=f"ib{qh}_{h}_{j}")
                    nc.gpsimd.partition_broadcast(invb[:], inv[:])
                    qsl = slice(1024 * qh + 512 * j, 1024 * qh + 512 * j + 512)
                    nc.vector.tensor_mul(AT[:, h, qsl], pvsb[0:64, :], invb[:])

            def out_proj_chunk(qc):
                """output projection for query rows [128*qc, 128*qc+128)."""
                o = opool.tile([128, D], bf16, tag="o", name=f"o{qc}")
                for nlo, nw in ((0, 512), (512, 256)):
                    ps_o = pp.tile([128, 512], f32, tag="pp",
                                   name=f"po{qc}_{nlo}")
                    for h in range(HP):
                        nc.tensor.matmul(
                            ps_o[:, 0:nw],
                            lhsT=AT[:, h, 128 * qc:128 * qc + 128],
                            rhs=wo[:, h, nlo:nlo + nw],
                            start=(h == 0), stop=(h == HP - 1),
                        )
                    nc.vector.tensor_copy(o[:, nlo:nlo + nw], ps_o[:, 0:nw])
                nc.sync.dma_start(
                    out=out_d[128 * qc:128 * qc + 128, :], in_=o[:])

            def head_ops(h):
                if h == 0:
                    return (lambda kc: KTa[0:64, 128 * kc:128 * kc + 128],
                            lambda lo: QTa[0:64, lo:lo + 512])
                if h == 1:
                    return (lambda kc: KTa[64:128, 128 * kc:128 * kc + 128],
                            lambda lo: QTa[64:128, lo:lo + 512])
                return (lambda kc: KTb[64:128, 128 * kc:128 * kc + 128],
                        lambda lo: QTb[64:128, lo:lo + 512])

            # ---- fused attention, transposed orientation, q-half major ----
            qh0_chunks = iter(range(8))
            for qh in range(2):
                ha = 1024 * qh
                for h in range(HP):
                    kslice, qslice = head_ops(h)
                    pvs = [pp.tile([65, 512], f32, tag="pp",
                                   name=f"pv{qh}_{h}_{j}") for j in range(2)]
                    if qh == 0 and h == 0:
                        v_proj(0)
                        v_proj(1)
                    for kc in range(KC):
                        s = sp.tile([128, 1024], f32, tag="sp",
                                    name=f"s{qh}_{h}_{kc}")
                        for jj in range(2):
                            nc.tensor.matmul(
                                s[:, 512 * jj:512 * jj + 512],
                                lhsT=kslice(kc), rhs=qslice(ha + 512 * jj),
                                start=True, stop=True)
                        band_add(s, h, kc, ha)
                        p = exp_split(s, h, kc, ha, f"p{qh}_{h}_{kc}")
                        for jj in range(2):
                            nc.tensor.matmul(
                                pvs[jj][:],
                                lhsT=Vg[:, kc, h, :],
                                rhs=p[:, 512 * jj:512 * jj + 512],
                                start=(kc == 0), stop=(kc == KC - 1))
                        # PE filler between the dependency-chained attention
                        # matmuls: V-projection chunks during q-half 0's
                        # first head, q-half 0's output projection during
                        # q-half 1
                        if qh == 0 and h == 0 and kc + 2 < KC:
                            v_proj(kc + 2)
                        if qh == 1 and kc % 4 == 3:
                            qc = next(qh0_chunks, None)
                            if qc is not None:
                                out_proj_chunk(qc)
                    normalize(pvs, h, qh)

            # ---- remaining output projection (q-half 1) ----
            for qc in range(8, L // 128):
                out_proj_chunk(qc)

    nc.compile()
    return nc


def _get_program():
    with _lock:
        if "nc" not in _cache:
            _cache["nc"] = _build_program()
        return _cache["nc"]


def _host_prep(core, query, key_value, key_padding_mask, Wq, Wk, Wv, Wo, rel_emb):
    import ml_dtypes

    bf16 = ml_dtypes.bfloat16
    b, g = core // 4, core % 4
    mask = key_padding_mask[b].astype(np.float32)
    kv = key_value[b] * mask[:, None]
    qT = np.ascontiguousarray(query[b].T).astype(bf16)
    kvT = np.ascontiguousarray(kv.T).astype(bf16)
    sl = slice(HD * g, HD * (g + 1))
    wq = np.ascontiguousarray(Wq[:, sl])
    wk = np.ascontiguousarray(Wk[:, sl]) * np.float32(DK ** -0.5)
    wv = np.ascontiguousarray(Wv[:, sl]).astype(bf16)
    wo = np.ascontiguousarray(
        Wo[sl].reshape(HP, 64, D).transpose(1, 0, 2)).astype(bf16)

    d = np.arange(-2047, 2048)
    buckets = _np_bucket(d)
    heads = [HP * g + i for i in range(HP)]
    t = rel_emb[buckets][:, heads].astype(np.float32)  # [4095, HP]
    cm = t[0]
    cp = t[-1]
    # sh[h, p, y] = t[y + 1793 + p, h] - cm[h]
    p_i = np.arange(128)[:, None]
    y_i = np.arange(383)[None, :]
    sh = np.ascontiguousarray(
        (t[y_i + 1793 + p_i] - cm[None, None, :]).transpose(2, 0, 1))
    msk = np.ascontiguousarray(mask.reshape(KC, 128).T)
    cmc = np.ascontiguousarray(np.broadcast_to(cm[None, :], (128, HP)))
    cpc = np.ascontiguousarray(np.broadcast_to(cp[None, :], (128, HP)))
    return {
        "qT": qT, "kvT": kvT,
        "wqa": np.ascontiguousarray(wq[:, 0:128]).astype(bf16),
        "wka": np.ascontiguousarray(wk[:, 0:128]).astype(bf16),
        "wqb": np.ascontiguousarray(wq[:, 128:192]).astype(bf16),
        "wkb": np.ascontiguousarray(wk[:, 128:192]).astype(bf16),
        "wv": wv, "wo": wo,
        "sh": sh.astype(np.float32), "msk": msk,
        "cm": cmc.astype(np.float32), "cp": cpc.astype(np.float32),
    }


def make_in_maps(**inputs):
    return [_host_prep(c, **inputs) for c in range(NCORES)]


def kernel(query, key_value, key_padding_mask, Wq, Wk, Wv, Wo, rel_emb,
           _results_hook=None, _run_kwargs=None):
    from concourse.bass_utils import run_bass_kernel_spmd

    inputs = dict(query=np.asarray(query), key_value=np.asarray(key_value),
                  key_padding_mask=np.asarray(key_padding_mask),
                  Wq=np.asarray(Wq, np.float32), Wk=np.asarray(Wk, np.float32),
                  Wv=np.asarray(Wv, np.float32), Wo=np.asarray(Wo, np.float32),
                  rel_emb=np.asarray(rel_emb, np.float32))
    nc = _get_program()
    in_maps = make_in_maps(**inputs)
    res = run_bass_kernel_spmd(nc, in_maps, core_ids=list(range(NCORES)),
                               **(_run_kwargs or {}))
    if _results_hook is not None:
        _results_hook(res)
    out = np.zeros((B, L, D), np.float32)
    for c in range(NCORES):
        out[c // 4] += res.results[c]["out_p"].astype(np.float32)
    return out


# revision 13
# speedup vs baseline: 1.0688x; 1.0489x over previous
"""Trainium2 Bass kernel for nn_MultiHeadAttention_44908178047033.

T5-style MHA (relative-position bias, bidirectional) over
B=2, L=2048, D=768, H=12, DK=64.

Sharding: 8 cores = 2 batches x 4 head-groups (3 heads each).
Each core computes Q/K/V projections for its (batch, 3 heads), fused
transposed-orientation attention (scores kept as S^T [k, q] so the
softmax denominator and the PV contraction both run as PE matmuls
without transposing the probability matrix), and a partial output
projection. Host sums the 4 per-head-group partials per batch.

v3 perf structure (from trace analysis of the f32r baseline and v2):
- everything bf16: halves input DMA, runs all matmuls at bf16 rate
- single ACT table preload (natural_log_exp_and_others) so the Ln/Exp
  softmax normalization never swaps activation tables mid-kernel
- q-half-major loop with sequential heads: only 2 PV accumulator banks
  and 2 double-buffered score tiles are live, leaving 2 PSUM banks for
  interleaved filler matmuls
- the PE HAM clock gate re-throttles to 1.2 GHz after any ~3.4us idle
  window and only re-warms after ~3.4us of continuous work, so the V
  projection is interleaved into q-half 0's attention and the output
  projection of q-half 0 into q-half 1's attention: the PE instruction
  queue never drains at phase transitions and stays at 2.4 GHz

Relative-position bias: the T5 bias f(k-q) is constant for |k-q| >= 128
(log-bucketing saturates), so
  exp(s + f) = exp(s + cm)            for k-q <= -128  (ACT bias, free)
             = exp(s + cp)            for k-q >= +128  (ACT bias, free)
             = exp(s + cm + (f - cm)) for |k-q| < 128  (DVE add from a
               host-precomputed per-partition shifted Toeplitz table,
               read with a negative free-dim stride)
"""

import math
import sys
import threading

import numpy as np

sys.path.insert(0, "/opt/trn_rl_repo")

B, L, D = 2, 2048, 768
H, DK = 12, 64
NUM_BUCKETS, MAX_DIST = 32, 128
HP = 3            # heads per core
HD = HP * DK      # 192 cols per head-group
NCORES = 8
KC = 16           # key chunks of 128
CCH = 6           # contraction chunks of 128 over D

_cache = {}
_lock = threading.Lock()


def _np_bucket(d):
    rel = d
    ret = np.zeros_like(rel)
    n = -rel
    nb = NUM_BUCKETS // 2
    ret = ret + (n < 0).astype(np.int32) * nb
    n = np.abs(n)
    mx = nb // 2
    is_small = n < mx
    n_safe = np.maximum(n, 1).astype(np.float32)
    vl = mx + (
        np.log(n_safe / mx) / math.log(MAX_DIST / mx) * (nb - mx)
    ).astype(np.int32)
    vl = np.minimum(vl, nb - 1)
    return ret + np.where(is_small, n, vl)


def _build_program():
    import concourse.bacc as bacc
    import concourse.bass as bass
    import concourse.mybir as mybir
    import concourse.tile as tile
    from concourse.hw_specs import get_activation_tables

    dt = mybir.dt
    f32, bf16 = dt.float32, dt.bfloat16
    Exp, Ln = mybir.ActivationFunctionType.Exp, mybir.ActivationFunctionType.Ln

    act_sets = list(get_activation_tables("gen3").keys())
    nle_id = act_sets.index("natural_log_exp_and_others")

    nc = bacc.Bacc("TRN2", target_bir_lowering=False, debug=False,
                   num_devices=NCORES)

    qT_d = nc.dram_tensor("qT", [D, L], bf16, kind="ExternalInput").ap()
    kvT_d = nc.dram_tensor("kvT", [D, L], bf16, kind="ExternalInput").ap()
    wqa_d = nc.dram_tensor("wqa", [D, 128], bf16, kind="ExternalInput").ap()
    wka_d = nc.dram_tensor("wka", [D, 128], bf16, kind="ExternalInput").ap()
    wqb_d = nc.dram_tensor("wqb", [D, 64], bf16, kind="ExternalInput").ap()
    wkb_d = nc.dram_tensor("wkb", [D, 64], bf16, kind="ExternalInput").ap()
    wv_d = nc.dram_tensor("wv", [D, HD], bf16, kind="ExternalInput").ap()
    wo_d = nc.dram_tensor("wo", [64, HP, D], bf16, kind="ExternalInput").ap()
    sh_d = nc.dram_tensor("sh", [HP, 128, 383], f32, kind="ExternalInput").ap()
    msk_d = nc.dram_tensor("msk", [128, KC], f32, kind="ExternalInput").ap()
    cm_d = nc.dram_tensor("cm", [128, HP], f32, kind="ExternalInput").ap()
    cp_d = nc.dram_tensor("cp", [128, HP], f32, kind="ExternalInput").ap()
    out_d = nc.dram_tensor("out_p", [L, D], bf16, kind="ExternalOutput").ap()

    with tile.TileContext(nc) as tc:
        with (
            tc.tile_pool(name="const", bufs=1) as cpool,
            tc.tile_pool(name="p", bufs=3) as ppool,
            tc.tile_pool(name="o", bufs=2) as opool,
            tc.tile_pool(name="nrm", bufs=4) as npool,
            tc.tile_pool(name="sp", bufs=2, space="PSUM") as sp,
            tc.tile_pool(name="pp", bufs=4, space="PSUM") as pp,
        ):
            # ---- persistent SBUF ----
            wqa = cpool.tile([128, CCH, 128], bf16, tag="wqa")
            wka = cpool.tile([128, CCH, 128], bf16, tag="wka")
            wqb = cpool.tile([128, CCH, 64], bf16, tag="wqb")
            wkb = cpool.tile([128, CCH, 64], bf16, tag="wkb")
            wv = cpool.tile([128, CCH, HD], bf16, tag="wv")
            wo = cpool.tile([64, HP, D], bf16, tag="wo")
            sh = cpool.tile([128, HP, 383], f32, tag="sh")
            msk = cpool.tile([128, KC], f32, tag="msk")
            cmc = cpool.tile([128, HP], f32, tag="cmc")
            cpc = cpool.tile([128, HP], f32, tag="cpc")
            qT = cpool.tile([128, CCH, L], bf16, tag="qT")
            kvT = cpool.tile([128, CCH, L], bf16, tag="kvT")
            # heads 0,1 stacked on partitions 0-63 / 64-127
            QTa = cpool.tile([128, L], bf16, tag="QTa")
            KTa = cpool.tile([128, L], bf16, tag="KTa")
            # head 2: K on partitions 64-127 (straight from its col-tiled
            # projection), Q on 0-63 then DMA-duplicated to 64-127 so both
            # score operands live on the same partition half
            QTb = cpool.tile([128, L], bf16, tag="QTb")
            KTb = cpool.tile([128, L], bf16, tag="KTb")
            Vg = cpool.tile([128, KC, HP, 65], bf16, tag="Vg")
            AT = cpool.tile([64, HP, L], bf16, tag="AT")

            # single activation-table load covering both Exp and Ln; the
            # compile-time pass then sees every activation's table resident
            nc.scalar.add_instruction(mybir.InstLoadActFuncSet(
                name=nc.get_next_instruction_name(), ins=[], outs=[],
                act_func_set_id=nle_id))

            # ---- loads (weights first on the scalar HWDGE path so the
            # first projection matmuls can start early; bulk activations
            # stream on sync) ----
            nc.scalar.dma_start(out=wqa[:], in_=wqa_d.rearrange("(c p) n -> p c n", p=128))
            nc.scalar.dma_start(out=wka[:], in_=wka_d.rearrange("(c p) n -> p c n", p=128))
            nc.scalar.dma_start(out=wqb[:], in_=wqb_d.rearrange("(c p) n -> p c n", p=128))
            nc.scalar.dma_start(out=wkb[:], in_=wkb_d.rearrange("(c p) n -> p c n", p=128))
            nc.scalar.dma_start(out=wv[:], in_=wv_d.rearrange("(c p) n -> p c n", p=128))
            nc.scalar.dma_start(out=wo[:], in_=wo_d)
            nc.scalar.dma_start(out=sh[:], in_=sh_d.rearrange("h p y -> p h y"))
            nc.scalar.dma_start(out=msk[:], in_=msk_d)
            nc.scalar.dma_start(out=cmc[:], in_=cm_d)
            nc.scalar.dma_start(out=cpc[:], in_=cp_d)
            qT_r = qT_d.rearrange("(c p) n -> p c n", p=128)
            kvT_r = kvT_d.rearrange("(c p) n -> p c n", p=128)
            for src_r, dst in ((kvT_r, kvT), (qT_r, qT)):
                for c in range(CCH):
                    nc.sync.dma_start(out=dst[:, c, :], in_=src_r[:, c, :])

            # ---- Q/K projections (heads 0,1; head 2 is deferred into
            # q-half 0's head-1 attention phase as PE filler) ----
            for w_in, x_in, dst in ((wka, kvT, KTa), (wqa, qT, QTa)):
                for n in range(4):
                    nsl = slice(512 * n, 512 * n + 512)
                    ps = pp.tile([128, 512], f32, tag="pp", name=f"ps{dst.name}_{n}")
                    for c in range(CCH):
                        nc.tensor.matmul(
                            ps[:], lhsT=w_in[:, c, :], rhs=x_in[:, c, nsl],
                            start=(c == 0), stop=(c == CCH - 1),
                        )
                    nc.vector.tensor_copy(dst[:, nsl], ps[:])

            def qk2_proj(n):
                """head 2's Q and K projection n-chunk, col-tiled in one
                pass (Q on array cols 0-63, K on 64-127), separate PSUM
                banks so the accumulation groups' has_written clears stay
                apart."""
                nsl = slice(512 * n, 512 * n + 512)
                psq = pp.tile([128, 512], f32, tag="pp", name=f"psbq_{n}")
                psk = pp.tile([128, 512], f32, tag="pp", name=f"psbk_{n}")
                for c in range(CCH):
                    nc.tensor.matmul(
                        psq[0:64, :], lhsT=wqb[:, c, :], rhs=qT[:, c, nsl],
                        start=(c == 0), stop=(c == CCH - 1),
                    )
                    nc.tensor.matmul(
                        psk[64:128, :], lhsT=wkb[:, c, :], rhs=kvT[:, c, nsl],
                        start=(c == 0), stop=(c == CCH - 1),
                        tile_position=(0, 64),
                    )
                nc.vector.tensor_copy(QTb[0:64, nsl], psq[0:64, :])
                nc.vector.tensor_copy(KTb[64:128, nsl], psk[64:128, :])

            # mask column of V_aug, all key chunks at once
            mrep = bass.AP(msk[:].tensor, msk[:].offset,
                           [list(msk[:].ap[0]), [1, KC], [0, HP], [1, 1]])
            nc.vector.tensor_copy(Vg[:, :, :, 64:65], mrep)

            def v_proj(kc, h):
                """V projection chunk for one head -> Vg[:, kc, h, 0:64]
                (interleaved just-in-time into that head's q-half 0
                attention as PE filler work)."""
                ps_v = pp.tile([128, 512], f32, tag="pp", name=f"psv{kc}_{h}")
                for c in range(CCH):
                    nc.tensor.matmul(
                        ps_v[:, 0:64],
                        lhsT=kvT[:, c, 128 * kc:128 * kc + 128],
                        rhs=wv[:, c, 64 * h:64 * h + 64],
                        start=(c == 0), stop=(c == CCH - 1),
                    )
                nc.vector.tensor_copy(Vg[:, kc, h, 0:64], ps_v[:, 0:64])

            def band_add(s, h, kc, ha):
                """near-diagonal bias add (in place, PSUM); s covers
                columns [ha, ha+1024)."""
                qlo = max(0, 128 * kc - 128)
                qhi = min(L, 128 * kc + 255)
                x0 = (2047 + 128 * kc - qlo) - 1793
                a = max(qlo, ha)
                b = min(qhi, ha + 1024)
                if b > a:
                    sh_ap = sh[:, h, :]
                    rev = bass.AP(
                        sh_ap.tensor, sh_ap.offset + x0 - (a - qlo),
                        [list(sh_ap.ap[0]), [-1, b - a]],
                    )
                    nc.vector.tensor_add(
                        s[:, a - ha:b - ha], s[:, a - ha:b - ha], rev)

            def exp_split(s, h, kc, ha, name):
                """exp with region-split bias: cp for q < wcp, cm after."""
                p = ppool.tile([128, 1024], bf16, tag="p", name=name)
                wcp = max(0, 128 * kc - 128)
                wl = min(max(wcp - ha, 0), 1024)
                if wl > 0:
                    nc.scalar.activation(
                        p[:, 0:wl], s[:, 0:wl], Exp,
                        bias=cpc[:, h:h + 1], scale=1.0)
                if wl < 1024:
                    nc.scalar.activation(
                        p[:, wl:1024], s[:, wl:1024], Exp,
                        bias=cmc[:, h:h + 1], scale=1.0)
                return p

            def normalize(pvs, h, qh):
                """pvs: two [65, 512] PSUM accumulators (numerator rows
                0-63, denominator row 64) -> AT[:, h, qh*1024 : +1024]."""
                pvsb = npool.tile([65, 1024], bf16, tag="pvsb",
                                  name=f"pvsb{qh}_{h}")
                for j in range(2):
                    nc.vector.tensor_copy(pvsb[:, 512 * j:512 * j + 512],
                                          pvs[j][:])
                lns = npool.tile([1, 1024], f32, tag="lns", name=f"l{qh}_{h}")
                nc.scalar.activation(lns[:], pvsb[64:65, :], Ln)
                inv = npool.tile([1, 1024], bf16, tag="inv", name=f"i{qh}_{h}")
                nc.scalar.activation(inv[:], lns[:], Exp, scale=-1.0)
                invb = npool.tile([64, 1024], bf16, tag="invb",
                                  name=f"ib{qh}_{h}")
                nc.gpsimd.partition_broadcast(invb[:], inv[:])
                qsl = slice(1024 * qh, 1024 * qh + 1024)
                nc.vector.tensor_mul(AT[:, h, qsl], pvsb[0:64, :], invb[:])

            _ostate = {}

            def out_proj_unit(qc, nlo):
                """one output-projection unit: 3 accumulating matmuls for
                query rows [128*qc, +128), output cols [nlo, nlo+nw); the
                second unit of a row chunk completes the tile and DMAs it."""
                nw = 512 if nlo == 0 else 256
                if nlo == 0:
                    _ostate[qc] = opool.tile([128, D], bf16, tag="o",
                                             name=f"o{qc}")
                o = _ostate[qc]
                ps_o = pp.tile([128, 512], f32, tag="pp", name=f"po{qc}_{nlo}")
                for h in range(HP):
                    nc.tensor.matmul(
                        ps_o[:, 0:nw],
                        lhsT=AT[:, h, 128 * qc:128 * qc + 128],
                        rhs=wo[:, h, nlo:nlo + nw],
                        start=(h == 0), stop=(h == HP - 1),
                    )
                nc.vector.tensor_copy(o[:, nlo:nlo + nw], ps_o[:, 0:nw])
                if nlo != 0:
                    nc.sync.dma_start(
                        out=out_d[128 * qc:128 * qc + 128, :], in_=o[:])

            def out_proj_chunk(qc):
                out_proj_unit(qc, 0)
                out_proj_unit(qc, 512)

            def head_ops(h):
                if h == 0:
                    return (lambda kc: KTa[0:64, 128 * kc:128 * kc + 128],
                            lambda lo: QTa[0:64, lo:lo + 512])
                if h == 1:
                    return (lambda kc: KTa[64:128, 128 * kc:128 * kc + 128],
                            lambda lo: QTa[64:128, lo:lo + 512])
                return (lambda kc: KTb[64:128, 128 * kc:128 * kc + 128],
                        lambda lo: QTb[64:128, lo:lo + 512])

            # ---- fused attention, transposed orientation, q-half major ----
            # PE filler schedule keeps the tensor engine's queue from
            # draining (and its HAM clock gate from re-throttling):
            #   qh0/h: that head's V-projection chunks just-in-time
            #   qh0/h1: also head 2's deferred Q/K projection
            #   qh1: q-half 0's output projection in 3-matmul units
            qh0_units = [(qc, nlo) for qc in range(8) for nlo in (0, 512)]
            qh0_units = iter(qh0_units)
            for qh in range(2):
                ha = 1024 * qh
                for h in range(HP):
                    kslice, qslice = head_ops(h)
                    pvs = [pp.tile([65, 512], f32, tag="pp",
                                   name=f"pv{qh}_{h}_{j}") for j in range(2)]
                    if qh == 0:
                        v_proj(0, h)
                        v_proj(1, h)
                    for kc in range(KC):
                        s = sp.tile([128, 1024], f32, tag="sp",
                                    name=f"s{qh}_{h}_{kc}")
                        for jj in range(2):
                            nc.tensor.matmul(
                                s[:, 512 * jj:512 * jj + 512],
                                lhsT=kslice(kc), rhs=qslice(ha + 512 * jj),
                                start=True, stop=True)
                        band_add(s, h, kc, ha)
                        p = exp_split(s, h, kc, ha, f"p{qh}_{h}_{kc}")
                        for jj in range(2):
                            nc.tensor.matmul(
                                pvs[jj][:],
                                lhsT=Vg[:, kc, h, :],
                                rhs=p[:, 512 * jj:512 * jj + 512],
                                start=(kc == 0), stop=(kc == KC - 1))
                        if qh == 0 and kc + 2 < KC:
                            v_proj(kc + 2, h)
                        if qh == 0 and h == 1 and kc % 4 == 2:
                            qk2_proj(kc // 4)
                        if qh == 1 and (h * KC + kc) % 3 == 1:
                            unit = next(qh0_units, None)
                            if unit is not None:
                                out_proj_unit(*unit)
                    normalize(pvs, h, qh)
                    if qh == 0 and h == 1:
                        # head-2 scores contract on partitions 64-127; its
                        # Q was projected onto 0-63 during the h1 phase
                        nc.sync.dma_start(out=QTb[64:128, :], in_=QTb[0:64, :])

            # ---- remaining output projection (q-half 1) ----
            for qc in range(8, L // 128):
                out_proj_chunk(qc)

    nc.compile()
    return nc


def _get_program():
    with _lock:
        if "nc" not in _cache:
            _cache["nc"] = _build_program()
        return _cache["nc"]


def _host_prep(core, query, key_value, key_padding_mask, Wq, Wk, Wv, Wo, rel_emb):
    import ml_dtypes

    bf16 = ml_dtypes.bfloat16
    b, g = core // 4, core % 4
    mask = key_padding_mask[b].astype(np.float32)
    kv = key_value[b] * mask[:, None]
    qT = np.ascontiguousarray(query[b].T).astype(bf16)
    kvT = np.ascontiguousarray(kv.T).astype(bf16)
    sl = slice(HD * g, HD * (g + 1))
    wq = np.ascontiguousarray(Wq[:, sl])
    wk = np.ascontiguousarray(Wk[:, sl]) * np.float32(DK ** -0.5)
    wv = np.ascontiguousarray(Wv[:, sl]).astype(bf16)
    wo = np.ascontiguousarray(
        Wo[sl].reshape(HP, 64, D).transpose(1, 0, 2)).astype(bf16)

    d = np.arange(-2047, 2048)
    buckets = _np_bucket(d)
    heads = [HP * g + i for i in range(HP)]
    t = rel_emb[buckets][:, heads].astype(np.float32)  # [4095, HP]
    cm = t[0]
    cp = t[-1]
    # sh[h, p, y] = t[y + 1793 + p, h] - cm[h]
    p_i = np.arange(128)[:, None]
    y_i = np.arange(383)[None, :]
    sh = np.ascontiguousarray(
        (t[y_i + 1793 + p_i] - cm[None, None, :]).transpose(2, 0, 1))
    msk = np.ascontiguousarray(mask.reshape(KC, 128).T)
    cmc = np.ascontiguousarray(np.broadcast_to(cm[None, :], (128, HP)))
    cpc = np.ascontiguousarray(np.broadcast_to(cp[None, :], (128, HP)))
    return {
        "qT": qT, "kvT": kvT,
        "wqa": np.ascontiguousarray(wq[:, 0:128]).astype(bf16),
        "wka": np.ascontiguousarray(wk[:, 0:128]).astype(bf16),
        "wqb": np.ascontiguousarray(wq[:, 128:192]).astype(bf16),
        "wkb": np.ascontiguousarray(wk[:, 128:192]).astype(bf16),
        "wv": wv, "wo": wo,
        "sh": sh.astype(np.float32), "msk": msk,
        "cm": cmc.astype(np.float32), "cp": cpc.astype(np.float32),
    }


def make_in_maps(**inputs):
    return [_host_prep(c, **inputs) for c in range(NCORES)]


def kernel(query, key_value, key_padding_mask, Wq, Wk, Wv, Wo, rel_emb,
           _results_hook=None, _run_kwargs=None):
    from concourse.bass_utils import run_bass_kernel_spmd

    inputs = dict(query=np.asarray(query), key_value=np.asarray(key_value),
                  key_padding_mask=np.asarray(key_padding_mask),
                  Wq=np.asarray(Wq, np.float32), Wk=np.asarray(Wk, np.float32),
                  Wv=np.asarray(Wv, np.float32), Wo=np.asarray(Wo, np.float32),
                  rel_emb=np.asarray(rel_emb, np.float32))
    nc = _get_program()
    in_maps = make_in_maps(**inputs)
    res = run_bass_kernel_spmd(nc, in_maps, core_ids=list(range(NCORES)),
                               **(_run_kwargs or {}))
    if _results_hook is not None:
        _results_hook(res)
    out = np.zeros((B, L, D), np.float32)
    for c in range(NCORES):
        out[c // 4] += res.results[c]["out_p"].astype(np.float32)
    return out
